# revision 4
# baseline (speedup 1.0000x reference)
"""ChemConv Bass kernel for 8 TRN2 NeuronCores.

Math: the reference
    node_connection[a,f,i] = sum_n conn[a,n,f] * x[n,i]
    bond_score[a,o,f]      = sum_i node_connection[a,f,i] * pf[o,f,i]
    out[a,o] = sum_f bond_score[a,o,f]*bf[o,f,0] + sum_{f,c} bp[a,f,c]*bf[o,f,1+c]
collapses algebraically to one large matmul plus small ones:
    W[o,f,i]  = pf[o,f,i] * bf[o,f,0]
    Y[k=(f,n), o] = sum_i x[n,i] * W[o,f,i]          (tiny: 24576 x 64)
    out[a,o]  = sum_k conn2d[a,k] * Y[k,o] + sum_j bpT[j,a] * bf2[j,o]
where conn2d[a, (f,n)] is the memory-bound stream.

Sharding: atoms (dim a) row-slabs of 256 across 8 cores. Each core computes
out_T[o, a_slab] via PSUM accumulation over 192 K-chunks of 128.

conn is cast to bf16 host-side (rel err ~3e-3, well under the 2e-2 gate):
halves the HBM stream to 12.6 MB/core and runs the PE at 1 cycle/row at any
p-state (fp32 is 4 cycles/row; fp32r needs ap>=256 and full p-state).
Host packs conn as [128, KC*AS] so each DMA batch reads 16 KB contiguous per
partition; the whole stream rides the SP HWDGE ring (splitting across both
rings was measured slower: per-queue rate drops ~25% under contention).
Y is computed on device from bf16 x^T/W in 8-chunk PSUM groups (1 bank),
one batch ahead of the conn matmuls that consume it.
"""

import numpy as np
import ml_dtypes

import concourse.bass as bass
import concourse.tile as tile
from concourse import bacc, mybir
from concourse.bass_utils import run_bass_kernel_spmd

A = 2048
IN_DEPTH = 64
OUT_DEPTH = 64
F = 12
NCORES = 8
AS = A // NCORES          # 256 atoms per core
K = A * F                 # 24576 contraction length
KP = 128                  # K per matmul chunk (partition dim)
KC = K // KP              # 192 chunks
NBLK = A // KP            # 16 n-blocks per filter tap
KB = 2 * F                # bond-term contraction length (f,c) = 24
YG = 8                    # y chunks per PSUM bank group (8*64 = 512 = bank)

BF16 = mybir.dt.bfloat16
F32 = mybir.dt.float32
NP_BF16 = ml_dtypes.bfloat16

_cache = {}


def _build_nc(B=32, bufs=6, y_ring=8, taper=(16, 8, 4, 2, 2)):
    """Build the per-core kernel.

    B: K-chunks per DMA batch (B*AS*2 bytes per partition per transfer)
    bufs: conn stream-pool buffering depth
    taper: tail batch sizes (sum must equal B) so the final accumulating
           matmuls that gate the output copy start as early as possible
    """
    nc = bacc.Bacc("TRN2", target_bir_lowering=False, debug=False)

    conn_t = nc.dram_tensor("conn_t", [KP, KC * AS], BF16, kind="ExternalInput").ap()
    # bond_t [24, AS] and bf2 [24, O] packed side by side -> one DMA
    bpack = nc.dram_tensor("bpack", [KB, AS + OUT_DEPTH], F32, kind="ExternalInput").ap()
    # xT [64, A] and Wr [64, F*O] packed side by side -> one DMA (bf16)
    xw = nc.dram_tensor("xw", [IN_DEPTH, A + F * OUT_DEPTH], BF16,
                        kind="ExternalInput").ap()
    out_t = nc.dram_tensor("out_t", [OUT_DEPTH, AS], F32, kind="ExternalOutput").ap()

    assert sum(taper) == B
    batches = [B] * (KC // B - 1) + list(taper)
    assert sum(batches) == KC
    NG = KC // YG               # y groups total

    with tile.TileContext(nc) as tc:
        with (
            tc.tile_pool(name="const", bufs=1) as cpool,
            tc.tile_pool(name="ypool", bufs=y_ring) as ypool,
            tc.tile_pool(name="stream", bufs=bufs) as spool,
            tc.tile_pool(name="psum", bufs=1, space="PSUM") as ppool,
            tc.tile_pool(name="ypsum", bufs=2, space="PSUM") as ypp,
        ):
            # small input DMAs, packed, on the second HWDGE ring (ACT) so the
            # conn stream owns the SP ring from t=0
            bp_sb = cpool.tile([KB, AS + OUT_DEPTH], F32)
            nc.scalar.dma_start(bp_sb[:], bpack[:])
            xw_sb = cpool.tile([IN_DEPTH, A + F * OUT_DEPTH], BF16)
            nc.scalar.dma_start(xw_sb[:], xw[:])
            bond_sb = bp_sb[:, :AS]
            bf2_sb = bp_sb[:, AS:AS + OUT_DEPTH]
            xt_sb = xw_sb[:, :A]
            wr_sb = xw_sb[:, A:A + F * OUT_DEPTH]

            ygroups = {}

            def y_chunk_ap(kc):
                g, j = divmod(kc, YG)
                return ygroups[g][:, j * OUT_DEPTH:(j + 1) * OUT_DEPTH]

            def y_group(g):
                # Y[kc=(f,nb)] chunk = xT[:, nb-block].T @ Wr[:, f-block];
                # each chunk is consumed by exactly one conn matmul, so
                # groups live in a small ring (ypool bufs) not a flat 6 MB
                yps = ypp.tile([KP, YG * OUT_DEPTH], F32, tag="yps")
                for j in range(YG):
                    kc = g * YG + j
                    f, nb = divmod(kc, NBLK)
                    nc.tensor.matmul(
                        yps[:, j * OUT_DEPTH:(j + 1) * OUT_DEPTH],
                        xt_sb[:, nb * KP:(nb + 1) * KP],
                        wr_sb[:, f * OUT_DEPTH:(f + 1) * OUT_DEPTH],
                        start=(j == 0),
                        stop=(j == YG - 1),
                    )
                yt = ypool.tile([KP, YG * OUT_DEPTH], BF16, tag="y",
                                name=f"yt_{g}")
                nc.vector.tensor_copy(yt[:], yps[:])
                ygroups[g] = yt

            # issue the first two conn batch DMAs before anything else so
            # the SP ring streams from t=0
            ctiles = {}
            starts = []
            k0 = 0
            for bsz in batches:
                starts.append(k0)
                k0 += bsz
            pre_issue = 2

            def issue_conn(bt):
                bsz = batches[bt]
                ctile = spool.tile([KP, bsz * AS], BF16, tag="conn",
                                   name=f"conn_{bt}")
                nc.sync.dma_start(
                    ctile[:], conn_t[:, starts[bt] * AS:(starts[bt] + bsz) * AS])
                ctiles[bt] = ctile

            for bt in range(pre_issue):
                issue_conn(bt)

            yg_done = 0
            acc = ppool.tile([OUT_DEPTH, AS], F32, tag="acc")

            # bond term opens the PSUM accumulation group
            nc.tensor.matmul(acc[:], bf2_sb[:], bond_sb[:], start=True, stop=False)

            for bt, bsz in enumerate(batches):
                # y groups needed by batch bt+1 (lookahead), before this
                # batch's matmuls occupy PE
                need = min(NG, -(-(starts[min(bt + 1, len(batches) - 1)]
                                   + batches[min(bt + 1, len(batches) - 1)]) // YG))
                while yg_done < need:
                    y_group(yg_done)
                    yg_done += 1
                for b in range(bsz):
                    kc = starts[bt] + b
                    nc.tensor.matmul(
                        acc[:],
                        y_chunk_ap(kc),
                        ctiles[bt][:, b * AS:(b + 1) * AS],
                        start=False,
                        stop=(kc == KC - 1),
                    )
                # prefetch next batch's DMA
                nxt = bt + pre_issue
                if nxt < len(batches):
                    issue_conn(nxt)

            out_sb = spool.tile([OUT_DEPTH, AS], F32, tag="osb")
            nc.vector.tensor_copy(out_sb[:], acc[:])
            nc.sync.dma_start(out_t[:], out_sb[:])

    nc.compile()
    return nc


def _prep(node_property_tensor, connectivity_tensor, bond_property_tensor,
          property_filters, bond_filters):
    x = np.asarray(node_property_tensor, dtype=np.float32)
    conn = np.asarray(connectivity_tensor, dtype=np.float32)
    bp = np.asarray(bond_property_tensor, dtype=np.float32)
    pf = np.asarray(property_filters, dtype=np.float32)
    bf = np.asarray(bond_filters, dtype=np.float32)

    W = pf * bf[:, :, 0:1]                                # (O, F, I)
    wr = np.ascontiguousarray(W.transpose(2, 1, 0).reshape(IN_DEPTH, F * OUT_DEPTH))
    bf2 = np.ascontiguousarray(bf[:, :, 1:3].reshape(OUT_DEPTH, KB).T)  # (24, O)

    # conn packed per core: [p, (f, nb, a)] so each k-chunk is a contiguous
    # [128, AS] block in DRAM (16 KB/partition per 32-chunk DMA batch)
    # conn[a, n=nb*128+p, f] -> packed[p, f, nb, a]
    connb = conn.astype(NP_BF16)                          # (A, A, F)
    cview = connb.reshape(A, NBLK, KP, F)                 # [a, nb, p, f]
    cpack = cview.transpose(2, 3, 1, 0)                   # [p, f, nb, a]

    xwp = np.ascontiguousarray(
        np.concatenate([x.T, wr], axis=1)).astype(NP_BF16)  # (64, A + F*O)

    in_maps = []
    for c in range(NCORES):
        sl = slice(c * AS, (c + 1) * AS)
        bond_tc = bp[sl].reshape(AS, KB).T                # (24, AS)
        in_maps.append({
            "conn_t": np.ascontiguousarray(
                cpack[:, :, :, sl].reshape(KP, KC * AS)),
            "bpack": np.ascontiguousarray(
                np.concatenate([bond_tc, bf2], axis=1)),  # (24, AS + O)
            "xw": xwp,
        })
    return in_maps


def kernel(node_property_tensor, connectivity_tensor, bond_property_tensor,
           property_filters, bond_filters):
    in_maps = _prep(node_property_tensor, connectivity_tensor,
                    bond_property_tensor, property_filters, bond_filters)

    if "nc" not in _cache:
        _cache["nc"] = _build_nc()
    nc = _cache["nc"]

    res = run_bass_kernel_spmd(nc, in_maps, core_ids=list(range(NCORES)))

    out = np.empty((A, OUT_DEPTH), dtype=np.float32)
    for c in range(NCORES):
        out[c * AS:(c + 1) * AS, :] = res.results[c]["out_t"].T
    return out


# revision 5
# speedup vs baseline: 1.2468x; 1.2468x over previous
"""ChemConv Bass kernel for 8 TRN2 NeuronCores.

Math: the reference
    node_connection[a,f,i] = sum_n conn[a,n,f] * x[n,i]
    bond_score[a,o,f]      = sum_i node_connection[a,f,i] * pf[o,f,i]
    out[a,o] = sum_f bond_score[a,o,f]*bf[o,f,0] + sum_{f,c} bp[a,f,c]*bf[o,f,1+c]
reassociates to a two-stage contraction that never materializes the big
Y[k,o] intermediate:
    W[o,f,i]   = pf[o,f,i] * bf[o,f,0]
    G_f[i,a]   = sum_n x[n,i] * conn[a,n,f]      (PE stage 1, the 12.6 MB
                                                  conn stream is the moving
                                                  operand, x n-blocks are the
                                                  stationary)
    out_T[o,a] = sum_f W_f[i,o]^T G_f[i,a] + bond (PE stage 2: 12 tiny
                                                  matmuls, ap=256)
Stage 1 is exactly one accumulating matmul per conn chunk - the same PE cost
as the plain conn@Y form - but stage 2 replaces 192 Y-matmuls + 24 copies
with 12 matmuls + 12 copies, keeping the PE strictly under the DMA stream.

Sharding: atoms (dim a) row-slabs of 256 across 8 cores; no communication.

conn is cast to bf16 host-side (rel err ~2.7e-3, well under the 2e-2 gate):
halves the HBM stream to 12.6 MB/core and runs the PE at 1 cycle/row at any
p-state. Host packs conn as [128, (f, nb, a)] so each k-chunk is a
contiguous [128, AS] block and each DMA batch reads 16 KB contiguous per
partition; the whole stream rides the SP HWDGE ring (splitting across both
rings measures slower: per-queue rate drops ~25% under contention).
Stage-2 matmul for filter f is deferred until filter f+1's group boundary so
the PE (in-order queue) never stalls on the G PSUM->SBUF copy.
"""

import numpy as np
import ml_dtypes

import concourse.bass as bass
import concourse.tile as tile
from concourse import bacc, mybir
from concourse.bass_utils import run_bass_kernel_spmd

A = 2048
IN_DEPTH = 64
OUT_DEPTH = 64
F = 12
NCORES = 8
AS = A // NCORES          # 256 atoms per core
K = A * F                 # 24576 contraction length
KP = 128                  # K per matmul chunk (partition dim)
KC = K // KP              # 192 chunks
NBLK = A // KP            # 16 n-blocks per filter tap
KB = 2 * F                # bond-term contraction length (f,c) = 24

BF16 = mybir.dt.bfloat16
F32 = mybir.dt.float32
NP_BF16 = ml_dtypes.bfloat16

_cache = {}


def _build_nc(B=32, bufs=6, taper=(16, 8, 4, 2, 2)):
    """Build the per-core kernel.

    B: K-chunks per DMA batch (B*AS*2 bytes per partition per transfer)
    bufs: conn stream-pool buffering depth
    taper: tail batch sizes (sum must equal B) so the final accumulating
           matmuls that gate the output copy start as early as possible
    """
    nc = bacc.Bacc("TRN2", target_bir_lowering=False, debug=False)

    conn_t = nc.dram_tensor("conn_t", [KP, KC * AS], BF16, kind="ExternalInput").ap()
    # bond_t [24, AS] and bf2 [24, O] packed side by side -> one DMA
    bpack = nc.dram_tensor("bpack", [KB, AS + OUT_DEPTH], F32, kind="ExternalInput").ap()
    # x blocked [p, (nb, i)]: x[nb*128+p, i]
    xpack = nc.dram_tensor("xpack", [KP, NBLK * IN_DEPTH], BF16,
                           kind="ExternalInput").ap()
    # W reshaped [i, (f, o)]
    wpack = nc.dram_tensor("wpack", [IN_DEPTH, F * OUT_DEPTH], BF16,
                           kind="ExternalInput").ap()
    out_t = nc.dram_tensor("out_t", [OUT_DEPTH, AS], F32, kind="ExternalOutput").ap()

    assert sum(taper) == B
    batches = [B] * (KC // B - 1) + list(taper)
    assert sum(batches) == KC

    with tile.TileContext(nc) as tc:
        with (
            tc.tile_pool(name="const", bufs=1) as cpool,
            tc.tile_pool(name="stream", bufs=bufs) as spool,
            tc.tile_pool(name="gsb", bufs=2) as gpool,
            tc.tile_pool(name="acc", bufs=1, space="PSUM") as ppool,
            tc.tile_pool(name="gpsum", bufs=2, space="PSUM") as gpp,
        ):
            # small input DMAs on the second HWDGE ring (ACT) so the conn
            # stream owns the SP ring from t=0
            bp_sb = cpool.tile([KB, AS + OUT_DEPTH], F32)
            nc.scalar.dma_start(bp_sb[:], bpack[:])
            x_sb = cpool.tile([KP, NBLK * IN_DEPTH], BF16)
            nc.scalar.dma_start(x_sb[:], xpack[:])
            w_sb = cpool.tile([IN_DEPTH, F * OUT_DEPTH], BF16)
            nc.scalar.dma_start(w_sb[:], wpack[:])
            bond_sb = bp_sb[:, :AS]
            bf2_sb = bp_sb[:, AS:AS + OUT_DEPTH]

            ctiles = {}
            starts = []
            k0 = 0
            for bsz in batches:
                starts.append(k0)
                k0 += bsz
            pre_issue = 2

            def issue_conn(bt):
                bsz = batches[bt]
                ctile = spool.tile([KP, bsz * AS], BF16, tag="conn",
                                   name=f"conn_{bt}")
                nc.sync.dma_start(
                    ctile[:], conn_t[:, starts[bt] * AS:(starts[bt] + bsz) * AS])
                ctiles[bt] = ctile

            for bt in range(pre_issue):
                issue_conn(bt)

            acc = ppool.tile([OUT_DEPTH, AS], F32, tag="acc")
            # bond term opens the PSUM accumulation group
            nc.tensor.matmul(acc[:], bf2_sb[:], bond_sb[:], start=True, stop=False)

            gsb = {}
            gps = None

            def stage2(f):
                # acc[o, a] += W_f[i, o]^T @ G_f[i, a]
                nc.tensor.matmul(
                    acc[:],
                    w_sb[:, f * OUT_DEPTH:(f + 1) * OUT_DEPTH],
                    gsb[f][:],
                    start=False,
                    stop=(f == F - 1),
                )

            for bt, bsz in enumerate(batches):
                for b in range(bsz):
                    kc = starts[bt] + b
                    f, nb = divmod(kc, NBLK)
                    if nb == 0:
                        gps = gpp.tile([IN_DEPTH, AS], F32, tag="gps")
                    # G_f[i, a] += x_nb[p, i]^T @ conn_chunk[p, a]
                    nc.tensor.matmul(
                        gps[:],
                        x_sb[:, nb * IN_DEPTH:(nb + 1) * IN_DEPTH],
                        ctiles[bt][:, b * AS:(b + 1) * AS],
                        start=(nb == 0),
                        stop=(nb == NBLK - 1),
                    )
                    if nb == NBLK - 1:
                        g = gpool.tile([IN_DEPTH, AS], BF16, tag="g",
                                       name=f"g_{f}")
                        nc.vector.tensor_copy(g[:], gps[:])
                        gsb[f] = g
                        # deferred: by now filter f-1's copy has long landed,
                        # so this matmul never stalls the in-order PE queue
                        if f > 0:
                            stage2(f - 1)
                nxt = bt + pre_issue
                if nxt < len(batches):
                    issue_conn(nxt)

            stage2(F - 1)

            out_sb = spool.tile([OUT_DEPTH, AS], F32, tag="osb")
            nc.vector.tensor_copy(out_sb[:], acc[:])
            nc.sync.dma_start(out_t[:], out_sb[:])

    nc.compile()
    return nc


def _prep(node_property_tensor, connectivity_tensor, bond_property_tensor,
          property_filters, bond_filters):
    x = np.asarray(node_property_tensor, dtype=np.float32)
    conn = np.asarray(connectivity_tensor, dtype=np.float32)
    bp = np.asarray(bond_property_tensor, dtype=np.float32)
    pf = np.asarray(property_filters, dtype=np.float32)
    bf = np.asarray(bond_filters, dtype=np.float32)

    W = pf * bf[:, :, 0:1]                                # (O, F, I)
    wr = np.ascontiguousarray(
        W.transpose(2, 1, 0).reshape(IN_DEPTH, F * OUT_DEPTH))  # [i, (f, o)]
    bf2 = np.ascontiguousarray(bf[:, :, 1:3].reshape(OUT_DEPTH, KB).T)  # (24, O)

    # conn packed per core: [p, (f, nb, a)] so each k-chunk is a contiguous
    # [128, AS] block in DRAM (16 KB/partition per 32-chunk DMA batch)
    # conn[a, n=nb*128+p, f] -> packed[p, f, nb, a]
    connb = conn.astype(NP_BF16)                          # (A, A, F)
    cview = connb.reshape(A, NBLK, KP, F)                 # [a, nb, p, f]
    cpack = cview.transpose(2, 3, 1, 0)                   # [p, f, nb, a]

    xp = np.ascontiguousarray(
        x.reshape(NBLK, KP, IN_DEPTH).transpose(1, 0, 2)
        .reshape(KP, NBLK * IN_DEPTH)).astype(NP_BF16)    # [p, (nb, i)]
    wp = wr.astype(NP_BF16)

    in_maps = []
    for c in range(NCORES):
        sl = slice(c * AS, (c + 1) * AS)
        bond_tc = bp[sl].reshape(AS, KB).T                # (24, AS)
        in_maps.append({
            "conn_t": np.ascontiguousarray(
                cpack[:, :, :, sl].reshape(KP, KC * AS)),
            "bpack": np.ascontiguousarray(
                np.concatenate([bond_tc, bf2], axis=1)),  # (24, AS + O)
            "xpack": xp,
            "wpack": wp,
        })
    return in_maps


def kernel(node_property_tensor, connectivity_tensor, bond_property_tensor,
           property_filters, bond_filters):
    in_maps = _prep(node_property_tensor, connectivity_tensor,
                    bond_property_tensor, property_filters, bond_filters)

    if "nc" not in _cache:
        _cache["nc"] = _build_nc()
    nc = _cache["nc"]

    res = run_bass_kernel_spmd(nc, in_maps, core_ids=list(range(NCORES)))

    out = np.empty((A, OUT_DEPTH), dtype=np.float32)
    for c in range(NCORES):
        out[c * AS:(c + 1) * AS, :] = res.results[c]["out_t"].T
    return out


# revision 9
# speedup vs baseline: 1.4254x; 1.1433x over previous
"""ChemConv Bass kernel for 8 TRN2 NeuronCores.

Math: the reference
    node_connection[a,f,i] = sum_n conn[a,n,f] * x[n,i]
    bond_score[a,o,f]      = sum_i node_connection[a,f,i] * pf[o,f,i]
    out[a,o] = sum_f bond_score[a,o,f]*bf[o,f,0] + sum_{f,c} bp[a,f,c]*bf[o,f,1+c]
reassociates to a two-stage contraction that never materializes the big
Y[k,o] intermediate:
    W[o,f,i]   = pf[o,f,i] * bf[o,f,0]
    G_f[i,a]   = sum_n x[n,i] * conn[a,n,f]      (PE stage 1, the 12.6 MB
                                                  conn stream is the moving
                                                  operand, x n-blocks are the
                                                  stationary)
    out_T[o,a] = sum_f W_f[i,o]^T G_f[i,a] + bond (PE stage 2: 12 tiny
                                                  matmuls, ap=256)
Stage 1 is exactly one accumulating matmul per conn chunk - the same PE cost
as the plain conn@Y form - but stage 2 replaces 192 Y-matmuls + 24 copies
with 12 matmuls + 12 copies, keeping the PE strictly under the DMA stream.

Sharding: atoms (dim a) row-slabs of 256 across 8 cores; no communication.

conn is cast to bf16 host-side (rel err ~2.7e-3, well under the 2e-2 gate):
halves the HBM stream to 12.6 MB/core and runs the PE at 1 cycle/row at any
p-state. Host packs conn as [128, (f, nb, a)] so each k-chunk is a
contiguous [128, AS] block and each DMA batch reads 16 KB contiguous per
partition; the whole stream rides the SP HWDGE ring (splitting across both
rings measures slower: per-queue rate drops ~25% under contention).
Stage-2 matmul for filter f is deferred until filter f+1's group boundary so
the PE (in-order queue) never stalls on the G PSUM->SBUF copy.
"""

import numpy as np
import ml_dtypes

import concourse.bass as bass
import concourse.tile as tile
from concourse import bacc, mybir
from concourse.bass_utils import run_bass_kernel_spmd

A = 2048
IN_DEPTH = 64
OUT_DEPTH = 64
F = 12
NCORES = 8
AS = A // NCORES          # 256 atoms per core
K = A * F                 # 24576 contraction length
KP = 128                  # K per matmul chunk (partition dim)
KC = K // KP              # 192 chunks
NBLK = A // KP            # 16 n-blocks per filter tap
KB = 2 * F                # bond-term contraction length (f,c) = 24

BF16 = mybir.dt.bfloat16
F32 = mybir.dt.float32
NP_BF16 = ml_dtypes.bfloat16

_cache = {}


def _build_nc(B=32, bufs=7, taper=(16, 8, 4, 2, 2)):
    """Build the per-core kernel.

    B: K-chunks per DMA batch (B*AS*2 bytes per partition per transfer)
    bufs: conn stream-pool buffering depth
    taper: tail batch sizes (sum must equal B) so the final accumulating
           matmuls that gate the output copy start as early as possible
    """
    nc = bacc.Bacc("TRN2", target_bir_lowering=False, debug=False)

    conn_t = nc.dram_tensor("conn_t", [KP, KC * AS], BF16, kind="ExternalInput").ap()
    # bond_t [24, AS] and bf2 [24, O] packed side by side -> one DMA
    bpack = nc.dram_tensor("bpack", [KB, AS + OUT_DEPTH], F32, kind="ExternalInput").ap()
    # x blocked [p, (nb, i)]: x[nb*128+p, i]
    xpack = nc.dram_tensor("xpack", [KP, NBLK * IN_DEPTH], BF16,
                           kind="ExternalInput").ap()
    # W reshaped [i, (f, o)]
    wpack = nc.dram_tensor("wpack", [IN_DEPTH, F * OUT_DEPTH], BF16,
                           kind="ExternalInput").ap()
    out_t = nc.dram_tensor("out_t", [OUT_DEPTH, AS], F32, kind="ExternalOutput").ap()

    assert sum(taper) == B
    batches = [B] * (KC // B - 1) + list(taper)
    assert sum(batches) == KC

    with tile.TileContext(nc) as tc:
        with (
            tc.tile_pool(name="const", bufs=1) as cpool,
            tc.tile_pool(name="stream", bufs=bufs) as spool,
            tc.tile_pool(name="gsb", bufs=2) as gpool,
            tc.tile_pool(name="acc", bufs=1, space="PSUM") as ppool,
            tc.tile_pool(name="gpsum", bufs=2, space="PSUM") as gpp,
        ):
            # x and W gate the first PE matmuls: put them FIRST on the SP
            # ring (0.8 us ahead of the conn stream) so PE can start the
            # moment conn batch 0 lands. bpack is only needed by the bond
            # matmul (deferred to the f=0 boundary) -> ACT ring, whose first
            # transfer starts ~3 us later than SP's.
            x_sb = cpool.tile([KP, NBLK * IN_DEPTH], BF16)
            nc.sync.dma_start(x_sb[:], xpack[:])
            w_sb = cpool.tile([IN_DEPTH, F * OUT_DEPTH], BF16)
            nc.sync.dma_start(w_sb[:], wpack[:])
            bp_sb = cpool.tile([KB, AS + OUT_DEPTH], F32)
            nc.scalar.dma_start(bp_sb[:], bpack[:])
            bond_sb = bp_sb[:, :AS]
            bf2_sb = bp_sb[:, AS:AS + OUT_DEPTH]

            ctiles = {}
            starts = []
            k0 = 0
            for bsz in batches:
                starts.append(k0)
                k0 += bsz
            pre_issue = 2

            def issue_conn(bt):
                bsz = batches[bt]
                ctile = spool.tile([KP, bsz * AS], BF16, tag="conn",
                                   name=f"conn_{bt}")
                nc.sync.dma_start(
                    ctile[:], conn_t[:, starts[bt] * AS:(starts[bt] + bsz) * AS])
                ctiles[bt] = ctile

            for bt in range(pre_issue):
                issue_conn(bt)

            acc = ppool.tile([OUT_DEPTH, AS], F32, tag="acc")
            gsb = {}
            gps = None

            def stage2(f):
                # acc[o, a] += W_f[i, o]^T @ G_f[i, a]
                nc.tensor.matmul(
                    acc[:],
                    w_sb[:, f * OUT_DEPTH:(f + 1) * OUT_DEPTH],
                    gsb[f][:],
                    start=False,
                    stop=(f == F - 1),
                )

            for bt, bsz in enumerate(batches):
                for b in range(bsz):
                    kc = starts[bt] + b
                    f, nb = divmod(kc, NBLK)
                    if nb == 0:
                        gps = gpp.tile([IN_DEPTH, AS], F32, tag="gps")
                    # G_f[i, a] += x_nb[p, i]^T @ conn_chunk[p, a]
                    nc.tensor.matmul(
                        gps[:],
                        x_sb[:, nb * IN_DEPTH:(nb + 1) * IN_DEPTH],
                        ctiles[bt][:, b * AS:(b + 1) * AS],
                        start=(nb == 0),
                        stop=(nb == NBLK - 1),
                    )
                    if nb == NBLK - 1:
                        g = gpool.tile([IN_DEPTH, AS], BF16, tag="g",
                                       name=f"g_{f}")
                        nc.vector.tensor_copy(g[:], gps[:])
                        gsb[f] = g
                        if f == 0:
                            # bond term opens the PSUM accumulation group;
                            # deferred to here so the in-order PE queue never
                            # stalls on the late bpack DMA
                            nc.tensor.matmul(acc[:], bf2_sb[:], bond_sb[:],
                                             start=True, stop=False)
                        # deferred: by now filter f-1's copy has long landed,
                        # so this matmul never stalls the in-order PE queue
                        if f > 0:
                            stage2(f - 1)
                nxt = bt + pre_issue
                if nxt < len(batches):
                    issue_conn(nxt)

            stage2(F - 1)

            out_sb = spool.tile([OUT_DEPTH, AS], F32, tag="osb")
            nc.vector.tensor_copy(out_sb[:], acc[:])
            nc.sync.dma_start(out_t[:], out_sb[:])

    nc.compile()
    return nc


def _prep(node_property_tensor, connectivity_tensor, bond_property_tensor,
          property_filters, bond_filters):
    x = np.asarray(node_property_tensor, dtype=np.float32)
    conn = np.asarray(connectivity_tensor, dtype=np.float32)
    bp = np.asarray(bond_property_tensor, dtype=np.float32)
    pf = np.asarray(property_filters, dtype=np.float32)
    bf = np.asarray(bond_filters, dtype=np.float32)

    W = pf * bf[:, :, 0:1]                                # (O, F, I)
    wr = np.ascontiguousarray(
        W.transpose(2, 1, 0).reshape(IN_DEPTH, F * OUT_DEPTH))  # [i, (f, o)]
    bf2 = np.ascontiguousarray(bf[:, :, 1:3].reshape(OUT_DEPTH, KB).T)  # (24, O)

    # conn packed per core: [p, (f, nb, a)] so each k-chunk is a contiguous
    # [128, AS] block in DRAM (16 KB/partition per 32-chunk DMA batch)
    # conn[a, n=nb*128+p, f] -> packed[p, f, nb, a]
    connb = conn.astype(NP_BF16)                          # (A, A, F)
    cview = connb.reshape(A, NBLK, KP, F)                 # [a, nb, p, f]
    cpack = cview.transpose(2, 3, 1, 0)                   # [p, f, nb, a]

    xp = np.ascontiguousarray(
        x.reshape(NBLK, KP, IN_DEPTH).transpose(1, 0, 2)
        .reshape(KP, NBLK * IN_DEPTH)).astype(NP_BF16)    # [p, (nb, i)]
    wp = wr.astype(NP_BF16)

    in_maps = []
    for c in range(NCORES):
        sl = slice(c * AS, (c + 1) * AS)
        bond_tc = bp[sl].reshape(AS, KB).T                # (24, AS)
        in_maps.append({
            "conn_t": np.ascontiguousarray(
                cpack[:, :, :, sl].reshape(KP, KC * AS)),
            "bpack": np.ascontiguousarray(
                np.concatenate([bond_tc, bf2], axis=1)),  # (24, AS + O)
            "xpack": xp,
            "wpack": wp,
        })
    return in_maps


def kernel(node_property_tensor, connectivity_tensor, bond_property_tensor,
           property_filters, bond_filters):
    in_maps = _prep(node_property_tensor, connectivity_tensor,
                    bond_property_tensor, property_filters, bond_filters)

    if "nc" not in _cache:
        _cache["nc"] = _build_nc()
    nc = _cache["nc"]

    res = run_bass_kernel_spmd(nc, in_maps, core_ids=list(range(NCORES)))

    out = np.empty((A, OUT_DEPTH), dtype=np.float32)
    for c in range(NCORES):
        out[c * AS:(c + 1) * AS, :] = res.results[c]["out_t"].T
    return out
